# revision 19
# baseline (speedup 1.0000x reference)
"""CapsuleFC routing kernel for 8 Trainium2 NeuronCores.

Problem (B, N, A, M, D) = (64, 128, 64, 128, 64):
    votes  = einsum('bna,namd->bnmd', input, w)
    logits = einsum('bnmd,bmd->bnm', votes, ncv) * (1/sqrt(D))
    qk     = softmax(logits, axis=m) * next_act ; renormalized over m
    ncv'   = einsum('bnm,bnmd,bn->bmd', qk, votes, current_act)

Sharding: input capsules n are split 16-per-core (w is read exactly once
across the system).  Softmax over m is per-(b, n), so every stage is
core-local; the only cross-core reduction is the final sum over n of the
ncv' partials, done on host (8 tiny (64,128,64) adds).

Per-core layout: capsules are processed in 8 pairs.  SBUF/PSUM tensors use
partitions = (pair parity, batch) = 2*64 = 128, free = (m, d) = 8192.
votes come from TensorE matmuls (lhsT = input slice (a=64, b=64), moving =
w slice), packed 4 capsules at a time into the 128x128 PE array via
tile_position row/col groups.  ScalarE evicts PSUM->SBUF as fp16; VectorE
does the logits multiply + d-reduction tree, the (folded) softmax, and the
qk-weighted accumulation, all in fp16 2x mode.
"""

import functools
import sys

import numpy as np

if "/opt/trn_rl_repo" not in sys.path:
    sys.path.insert(0, "/opt/trn_rl_repo")

B, N, A, M, D = 64, 128, 64, 128, 64
NCORES = 8
NLOC = N // NCORES   # 16 capsules per core
NPAIR = NLOC // 2    # 8
NGRP = NLOC // 4     # 4 (matmul packing groups of 4 capsules)
F = M * D            # 8192
SCALE = 1.0 / np.sqrt(D)
EXP_BIAS = -2.0      # constant shift inside exp; cancels in renormalization


@functools.lru_cache(maxsize=1)
def _build():
    import concourse.tile as tile
    from concourse import bacc, mybir
    import concourse.bass as bass
    from contextlib import ExitStack

    f16 = mybir.dt.float16
    f32 = mybir.dt.float32
    mult = mybir.AluOpType.mult
    add = mybir.AluOpType.add

    nc = bacc.Bacc("TRN2", target_bir_lowering=False, debug=False,
                   num_devices=NCORES)

    w_d = nc.dram_tensor("w", [NGRP, 2, 128, F], f16, kind="ExternalInput")
    in_d = nc.dram_tensor("inp", [128, NGRP, 2, B], f16, kind="ExternalInput")
    ncv_d = nc.dram_tensor("ncv", [B, F], f16, kind="ExternalInput")
    na_d = nc.dram_tensor("na", [B, M], f16, kind="ExternalInput")
    act_d = nc.dram_tensor("act", [128, NPAIR], f32, kind="ExternalInput")
    qk_d = nc.dram_tensor("qk", [128, NPAIR, M], f32, kind="ExternalOutput")
    acc_d = nc.dram_tensor("ncv_p", [128, F], f16, kind="ExternalOutput")

    with tile.TileContext(nc) as tc, ExitStack() as ctx:
        consts = ctx.enter_context(tc.tile_pool(name="consts", bufs=1))
        wpool = ctx.enter_context(tc.tile_pool(name="wpool", bufs=8))
        vpool = ctx.enter_context(tc.tile_pool(name="votes", bufs=4))
        ppool = ctx.enter_context(tc.tile_pool(name="prod", bufs=1))
        qpool = ctx.enter_context(tc.tile_pool(name="qe", bufs=2))
        tpool = ctx.enter_context(tc.tile_pool(name="tree", bufs=1))
        spool = ctx.enter_context(tc.tile_pool(name="small", bufs=2))
        psum = ctx.enter_context(tc.tile_pool(name="psum", bufs=2, space="PSUM"))

        # --- constants / whole-kernel tensors ---
        in_sb = consts.tile([128, NGRP, 2, B], f16)
        nc.sync.dma_start(out=in_sb[:], in_=in_d[:])
        ncv_sb = consts.tile([128, F], f16)
        na_sb = consts.tile([128, M], f16)
        act_sb = consts.tile([128, NPAIR], f32)

        def emit_const_dmas():
            nc.sync.dma_start(out=ncv_sb[0:64, :], in_=ncv_d[:, :])
            nc.sync.dma_start(out=ncv_sb[64:128, :], in_=ncv_d[:, :])
            nc.sync.dma_start(out=na_sb[0:64, :], in_=na_d[:, :])
            nc.sync.dma_start(out=na_sb[64:128, :], in_=na_d[:, :])
            nc.sync.dma_start(out=act_sb[:], in_=act_d[:])

        acc = consts.tile([128, F], f16)
        qk_sb = consts.tile([128, NPAIR, M], f32)
        exp_bias = consts.tile([128, 1], f32)
        nc.vector.memset(exp_bias[:], EXP_BIAS)

        w_tiles = {}
        CH = 2048           # psum tile free size (4 banks)
        NT = F // CH        # 4 psum tiles per pair

        def emit_w_dma(g):
            # one slab per (j, t): fine release granularity so the next
            # group's DMA streams in as this group's matmuls retire slabs
            for t in range(NT):
                for j in (0, 1):
                    s = wpool.tile([128, CH], f16, tag="w",
                                   name=f"w_{g}_{j}_{t}")
                    nc.sync.dma_start(
                        out=s[:], in_=w_d[g, j, :, t * CH:(t + 1) * CH])
                    w_tiles[(g, j, t)] = s

        votes_of = {}

        def emit_mms(p):
            """TensorE votes matmuls + ScalarE PSUM->SBUF eviction (E_p)."""
            g, s = p // 2, p % 2
            if s == 0 and (g, 0, 0) not in w_tiles:
                emit_w_dma(g)
            votes = vpool.tile([128, F], f16, tag="votes", name=f"votes_{p}")
            votes_of[p] = votes
            rows = slice(s * 64, s * 64 + 64)
            for t in range(NT):
                ps = psum.tile([128, CH], f32, tag="ps")
                w0, w1 = w_tiles[(g, 0, t)], w_tiles[(g, 1, t)]
                for cc in range(CH // 512):
                    col = cc * 512
                    nc.tensor.matmul(
                        ps[0:64, cc * 512:(cc + 1) * 512],
                        in_sb[rows, g, 0, :], w0[rows, col:col + 512],
                        start=True, stop=True, tile_position=(s * 64, 0))
                    nc.tensor.matmul(
                        ps[64:128, cc * 512:(cc + 1) * 512],
                        in_sb[rows, g, 1, :], w1[rows, col:col + 512],
                        start=True, stop=True, tile_position=(s * 64, 64))
                nc.scalar.copy(out=votes[:, t * CH:(t + 1) * CH], in_=ps[:])

        logit_of, e_of, sumexp_of, qk2_of, qe_of = {}, {}, {}, {}, {}

        def emit_logits(p):
            """L_p: votes*ncv multiply + d-reduction tree (VectorE)."""
            votes = votes_of[p]
            prod = ppool.tile([128, F], f16, tag="prod", name=f"prod_{p}")
            nq = 4 if p < 2 else 2
            Hq = F // nq
            for qq in range(nq):
                sl = slice(qq * Hq, (qq + 1) * Hq)
                nc.vector.tensor_mul(prod[:, sl], votes[:, sl], ncv_sb[:, sl])
            p3 = prod[:].rearrange("q (m d) -> q m d", d=D)
            t1 = tpool.tile([128, M, 32], f16, tag="t1")
            nc.vector.tensor_add(t1[:], p3[:, :, 0:32], p3[:, :, 32:64])
            t2 = tpool.tile([128, M, 16], f16, tag="t2")
            nc.vector.tensor_add(t2[:], t1[:, :, 0:16], t1[:, :, 16:32])
            t3 = tpool.tile([128, M, 8], f16, tag="t3")
            nc.vector.tensor_add(t3[:], t2[:, :, 0:8], t2[:, :, 8:16])
            t4 = tpool.tile([128, M, 4], f16, tag="t4")
            nc.vector.tensor_add(t4[:], t3[:, :, 0:4], t3[:, :, 4:8])
            t5 = tpool.tile([128, M, 2], f16, tag="t5")
            nc.vector.tensor_add(t5[:], t4[:, :, 0:2], t4[:, :, 2:4])
            logit = spool.tile([128, M], f32, tag="logit", name=f"logit_{p}")
            nc.vector.tensor_add(logit[:], t5[:, :, 0], t5[:, :, 1])
            logit_of[p] = logit

        def emit_exp(p):
            """X_p: exp with running sum (ScalarE)."""
            e = spool.tile([128, M], f16, tag="e", name=f"e_{p}")
            sumexp = spool.tile([128, 1], f32, tag="sumexp", name=f"se_{p}")
            nc.scalar.activation(
                out=e[:], in_=logit_of[p][:],
                func=mybir.ActivationFunctionType.Exp,
                bias=exp_bias[:], scale=float(SCALE), accum_out=sumexp[:])
            e_of[p], sumexp_of[p] = e, sumexp

        def emit_smalls(p):
            """S_p: folded softmax renorm, qk output, gated qk (VectorE)."""
            e, sumexp = e_of[p], sumexp_of[p]
            f_ = spool.tile([128, M], f16, tag="f", name=f"f_{p}")
            s_ = spool.tile([128, 1], f32, tag="s", name=f"s_{p}")
            nc.vector.tensor_mul(f_[:], e[:], na_sb[:])
            nc.vector.tensor_reduce(out=s_[:], in_=f_[:],
                                    axis=mybir.AxisListType.X, op=add)
            den = spool.tile([128, 1], f32, tag="den", name=f"den_{p}")
            nc.vector.scalar_tensor_tensor(
                out=den[:], in0=sumexp[:], scalar=1e-10, in1=s_[:],
                op0=mult, op1=add)
            r = spool.tile([128, 1], f32, tag="r", name=f"r_{p}")
            nc.vector.reciprocal(out=r[:], in_=den[:])
            nc.vector.tensor_scalar_mul(qk_sb[:, p, :], f_[:], r[:])
            qk2 = spool.tile([128, M], f16, tag="qk2", name=f"qk2_{p}")
            nc.vector.tensor_scalar_mul(qk2[:], qk_sb[:, p, :],
                                        act_sb[:, p:p + 1])
            qk2_of[p] = qk2

        def emit_qkexp(p):
            """Q_p: broadcast qk2 over d via a 0-stride AP (ScalarE)."""
            qe = qpool.tile([128, F], f16, tag="qe", name=f"qe_{p}")
            qa = qk2_of[p][:]
            qk2b = bass.AP(tensor=qa.tensor, offset=qa.offset,
                           ap=[qa.ap[0], qa.ap[1], [0, D]])
            nc.scalar.copy(out=qe[:].rearrange("q (m d) -> q m d", d=D),
                           in_=qk2b)
            qe_of[p] = qe

        def emit_ncv(p):
            """N_p: qk-weighted accumulate (VectorE)."""
            votes, qe = votes_of[p], qe_of[p]
            if p == 0:
                nc.vector.tensor_mul(acc[:], votes[:], qe[:])
            else:
                prod2 = ppool.tile([128, F], f16, tag="prod",
                                   name=f"prod2_{p}")
                nc.vector.tensor_mul(prod2[:], votes[:], qe[:])
                nc.vector.tensor_add(acc[:], acc[:], prod2[:])

        # software pipeline over pairs, depth 2:
        #   DVE FIFO per cycle c: [L_c, S_{c-1}, N_{c-2}]
        #   ACT FIFO per cycle c: [E_{c+1}, Q_{c-1}, X_c]
        # so the 7us qk-broadcast Q_{c-1} overlaps L_c, and N_{c-2}'s input
        # was produced a full cycle earlier.
        emit_mms(0)
        emit_const_dmas()
        emit_mms(1)
        for c in range(NPAIR + 2):
            if 2 <= c + 1 <= NPAIR - 1:
                emit_mms(c + 1)
            if 0 <= c - 1 <= NPAIR - 1:
                emit_smalls(c - 1)
                emit_qkexp(c - 1)
            if c <= NPAIR - 1:
                emit_logits(c)
                emit_exp(c)
            if c >= 2:
                emit_ncv(c - 2)

        nc.sync.dma_start(out=qk_d[:], in_=qk_sb[:])
        nc.sync.dma_start(out=acc_d[:], in_=acc[:])

    nc.compile()
    return nc


def _in_maps(input, current_act, next_capsule_value, next_act, w):
    """Build the 8 per-core input maps (all fp16, pre-laid-out)."""
    f16 = np.float16
    input = np.asarray(input, np.float32)
    current_act = np.asarray(current_act, np.float32)
    ncv = np.asarray(next_capsule_value, np.float32).reshape(B, F).astype(f16)
    na = np.asarray(next_act, np.float32).astype(f16)
    w = np.asarray(w, np.float32)

    maps = []
    for c in range(NCORES):
        ns = slice(NLOC * c, NLOC * (c + 1))
        # w: (NLOC, A, M, D) -> (NGRP, 2, 128, F); rows 0:64 hold capsule
        # 4g+j's (a, m*d) slab, rows 64:128 hold capsule 4g+2+j's.
        wr = w[ns].reshape(NGRP, 4, A, F).astype(f16)
        w_dev = np.empty((NGRP, 2, 128, F), f16)
        w_dev[:, 0, :64] = wr[:, 0]
        w_dev[:, 0, 64:] = wr[:, 2]
        w_dev[:, 1, :64] = wr[:, 1]
        w_dev[:, 1, 64:] = wr[:, 3]
        # input: lhsT slabs (a, b) per capsule, same 4-capsule packing
        it = input[:, ns, :].transpose(2, 1, 0).reshape(A, NGRP, 4, B).astype(f16)
        in_dev = np.empty((128, NGRP, 2, B), f16)
        in_dev[:64, :, 0] = it[:, :, 0]
        in_dev[:64, :, 1] = it[:, :, 1]
        in_dev[64:, :, 0] = it[:, :, 2]
        in_dev[64:, :, 1] = it[:, :, 3]
        # current_act: per-partition gate per pair, partitions = (parity, b)
        ca = current_act[:, ns]                      # (B, NLOC)
        act_dev = np.empty((128, NPAIR), np.float32)
        act_dev[:64] = ca[:, 0::2]
        act_dev[64:] = ca[:, 1::2]
        maps.append({"w": w_dev, "inp": in_dev, "ncv": ncv, "na": na,
                     "act": act_dev})
    return maps


def _execute(maps, trace=False, **kw):
    from concourse.bass_utils import run_bass_kernel_spmd
    nc = _build()
    return run_bass_kernel_spmd(nc, maps, core_ids=list(range(NCORES)),
                                trace=trace, **kw)


def _gather(results):
    qk_full = np.empty((B, N, M), np.float32)
    ncv_out = np.zeros((B, M, D), np.float32)
    for c in range(NCORES):
        qk_c = np.asarray(results[c]["qk"], np.float32)     # (128, NPAIR, M)
        # partitions = (parity, b); local capsule index = 2*pair + parity
        qk_r = qk_c.reshape(2, 64, NPAIR, M).transpose(1, 2, 0, 3)
        qk_full[:, NLOC * c:NLOC * (c + 1), :] = qk_r.reshape(B, NLOC, M)
        a = np.asarray(results[c]["ncv_p"], np.float32).reshape(2, 64, M, D)
        ncv_out += a[0] + a[1]
    na_out = np.ones((B, M), np.float32)
    return ncv_out, na_out, qk_full


def kernel(input, current_act, next_capsule_value, next_act, w, num_iter=1):
    maps = _in_maps(input, current_act, next_capsule_value, next_act, w)
    res = _execute(maps)
    return _gather(res.results)


# revision 20
# speedup vs baseline: 1.0371x; 1.0371x over previous
"""CapsuleFC routing kernel for 8 Trainium2 NeuronCores.

Problem (B, N, A, M, D) = (64, 128, 64, 128, 64):
    votes  = einsum('bna,namd->bnmd', input, w)
    logits = einsum('bnmd,bmd->bnm', votes, ncv) * (1/sqrt(D))
    qk     = softmax(logits, axis=m) * next_act ; renormalized over m
    ncv'   = einsum('bnm,bnmd,bn->bmd', qk, votes, current_act)

Sharding: input capsules n are split 16-per-core (w is read exactly once
across the system).  Softmax over m is per-(b, n), so every stage is
core-local; the only cross-core reduction is the final sum over n of the
ncv' partials, done on host (8 tiny (64,128,64) adds).

Per-core layout: capsules are processed in 8 pairs.  SBUF/PSUM tensors use
partitions = (pair parity, batch) = 2*64 = 128, free = (m, d) = 8192.
votes come from TensorE matmuls (lhsT = input slice (a=64, b=64), moving =
w slice), packed 4 capsules at a time into the 128x128 PE array via
tile_position row/col groups.  ScalarE evicts PSUM->SBUF as fp16; VectorE
does the logits multiply + d-reduction tree, the (folded) softmax, and the
qk-weighted accumulation, all in fp16 2x mode.
"""

import functools
import sys

import numpy as np

if "/opt/trn_rl_repo" not in sys.path:
    sys.path.insert(0, "/opt/trn_rl_repo")

B, N, A, M, D = 64, 128, 64, 128, 64
NCORES = 8
NLOC = N // NCORES   # 16 capsules per core
NPAIR = NLOC // 2    # 8
NGRP = NLOC // 4     # 4 (matmul packing groups of 4 capsules)
F = M * D            # 8192
SCALE = 1.0 / np.sqrt(D)
EXP_BIAS = -2.0      # constant shift inside exp; cancels in renormalization


@functools.lru_cache(maxsize=1)
def _build():
    import concourse.tile as tile
    from concourse import bacc, mybir
    import concourse.bass as bass
    from contextlib import ExitStack

    f16 = mybir.dt.float16
    f32 = mybir.dt.float32
    mult = mybir.AluOpType.mult
    add = mybir.AluOpType.add

    nc = bacc.Bacc("TRN2", target_bir_lowering=False, debug=False,
                   num_devices=NCORES)

    w_d = nc.dram_tensor("w", [NGRP, 2, 128, F], f16, kind="ExternalInput")
    in_d = nc.dram_tensor("inp", [128, NGRP, 2, B], f16, kind="ExternalInput")
    ncv_d = nc.dram_tensor("ncv", [128, F], f16, kind="ExternalInput")
    na_d = nc.dram_tensor("na", [128, M], f16, kind="ExternalInput")
    act_d = nc.dram_tensor("act", [128, NPAIR], f32, kind="ExternalInput")
    qk_d = nc.dram_tensor("qk", [128, NPAIR, M], f32, kind="ExternalOutput")
    acc_d = nc.dram_tensor("ncv_p", [128, F], f16, kind="ExternalOutput")

    with tile.TileContext(nc) as tc, ExitStack() as ctx:
        consts = ctx.enter_context(tc.tile_pool(name="consts", bufs=1))
        wpool = ctx.enter_context(tc.tile_pool(name="wpool", bufs=8))
        vpool = ctx.enter_context(tc.tile_pool(name="votes", bufs=4))
        ppool = ctx.enter_context(tc.tile_pool(name="prod", bufs=1))
        qpool = ctx.enter_context(tc.tile_pool(name="qe", bufs=2))
        tpool = ctx.enter_context(tc.tile_pool(name="tree", bufs=1))
        spool = ctx.enter_context(tc.tile_pool(name="small", bufs=2))
        psum = ctx.enter_context(tc.tile_pool(name="psum", bufs=2, space="PSUM"))

        # --- constants / whole-kernel tensors ---
        in_sb = consts.tile([128, NGRP, 2, B], f16)
        nc.sync.dma_start(out=in_sb[:], in_=in_d[:])
        ncv_sb = consts.tile([128, F], f16)
        na_sb = consts.tile([128, M], f16)
        act_sb = consts.tile([128, NPAIR], f32)

        def emit_const_dmas():
            H = F // 2
            nc.sync.dma_start(out=ncv_sb[:, :H], in_=ncv_d[:, :H])
            nc.sync.dma_start(out=ncv_sb[:, H:], in_=ncv_d[:, H:])
            nc.sync.dma_start(out=na_sb[:], in_=na_d[:])
            nc.sync.dma_start(out=act_sb[:], in_=act_d[:])

        acc = consts.tile([128, F], f16)
        qk_sb = consts.tile([128, NPAIR, M], f32)
        exp_bias = consts.tile([128, 1], f32)
        nc.vector.memset(exp_bias[:], EXP_BIAS)

        w_tiles = {}
        CH = 2048           # psum tile free size (4 banks)
        NT = F // CH        # 4 psum tiles per pair

        def emit_w_dma(g):
            # one slab per (j, t): fine release granularity so the next
            # group's DMA streams in as this group's matmuls retire slabs
            for t in range(NT):
                for j in (0, 1):
                    s = wpool.tile([128, CH], f16, tag="w",
                                   name=f"w_{g}_{j}_{t}")
                    nc.sync.dma_start(
                        out=s[:], in_=w_d[g, j, :, t * CH:(t + 1) * CH])
                    w_tiles[(g, j, t)] = s

        votes_of = {}

        def emit_mms(p):
            """TensorE votes matmuls + ScalarE PSUM->SBUF eviction (E_p)."""
            g, s = p // 2, p % 2
            if s == 0 and (g, 0, 0) not in w_tiles:
                emit_w_dma(g)
            votes = vpool.tile([128, F], f16, tag="votes", name=f"votes_{p}")
            votes_of[p] = votes
            rows = slice(s * 64, s * 64 + 64)
            for t in range(NT):
                ps = psum.tile([128, CH], f32, tag="ps")
                w0, w1 = w_tiles[(g, 0, t)], w_tiles[(g, 1, t)]
                for cc in range(CH // 512):
                    col = cc * 512
                    nc.tensor.matmul(
                        ps[0:64, cc * 512:(cc + 1) * 512],
                        in_sb[rows, g, 0, :], w0[rows, col:col + 512],
                        start=True, stop=True, tile_position=(s * 64, 0))
                    nc.tensor.matmul(
                        ps[64:128, cc * 512:(cc + 1) * 512],
                        in_sb[rows, g, 1, :], w1[rows, col:col + 512],
                        start=True, stop=True, tile_position=(s * 64, 64))
                nc.scalar.copy(out=votes[:, t * CH:(t + 1) * CH], in_=ps[:])

        logit_of, e_of, sumexp_of, qk2_of, qe_of = {}, {}, {}, {}, {}

        def emit_logits(p):
            """L_p: votes*ncv multiply + d-reduction tree (VectorE)."""
            votes = votes_of[p]
            prod = ppool.tile([128, F], f16, tag="prod", name=f"prod_{p}")
            H = F // 2
            nc.vector.tensor_mul(prod[:, :H], votes[:, :H], ncv_sb[:, :H])
            nc.vector.tensor_mul(prod[:, H:], votes[:, H:], ncv_sb[:, H:])
            p3 = prod[:].rearrange("q (m d) -> q m d", d=D)
            t1 = tpool.tile([128, M, 32], f16, tag="t1")
            nc.vector.tensor_add(t1[:], p3[:, :, 0:32], p3[:, :, 32:64])
            t2 = tpool.tile([128, M, 16], f16, tag="t2")
            nc.vector.tensor_add(t2[:], t1[:, :, 0:16], t1[:, :, 16:32])
            t3 = tpool.tile([128, M, 8], f16, tag="t3")
            nc.vector.tensor_add(t3[:], t2[:, :, 0:8], t2[:, :, 8:16])
            t4 = tpool.tile([128, M, 4], f16, tag="t4")
            nc.vector.tensor_add(t4[:], t3[:, :, 0:4], t3[:, :, 4:8])
            t5 = tpool.tile([128, M, 2], f16, tag="t5")
            nc.vector.tensor_add(t5[:], t4[:, :, 0:2], t4[:, :, 2:4])
            logit = spool.tile([128, M], f32, tag="logit", name=f"logit_{p}")
            nc.vector.tensor_add(logit[:], t5[:, :, 0], t5[:, :, 1])
            logit_of[p] = logit

        def emit_exp(p):
            """X_p: exp with running sum (ScalarE)."""
            e = spool.tile([128, M], f16, tag="e", name=f"e_{p}")
            sumexp = spool.tile([128, 1], f32, tag="sumexp", name=f"se_{p}")
            nc.scalar.activation(
                out=e[:], in_=logit_of[p][:],
                func=mybir.ActivationFunctionType.Exp,
                bias=exp_bias[:], scale=float(SCALE), accum_out=sumexp[:])
            e_of[p], sumexp_of[p] = e, sumexp

        def emit_smalls(p):
            """S_p: folded softmax renorm, qk output, gated qk (VectorE)."""
            e, sumexp = e_of[p], sumexp_of[p]
            f_ = spool.tile([128, M], f16, tag="f", name=f"f_{p}")
            s_ = spool.tile([128, 1], f32, tag="s", name=f"s_{p}")
            nc.vector.tensor_mul(f_[:], e[:], na_sb[:])
            nc.vector.tensor_reduce(out=s_[:], in_=f_[:],
                                    axis=mybir.AxisListType.X, op=add)
            den = spool.tile([128, 1], f32, tag="den", name=f"den_{p}")
            nc.vector.scalar_tensor_tensor(
                out=den[:], in0=sumexp[:], scalar=1e-10, in1=s_[:],
                op0=mult, op1=add)
            r = spool.tile([128, 1], f32, tag="r", name=f"r_{p}")
            nc.vector.reciprocal(out=r[:], in_=den[:])
            nc.vector.tensor_scalar_mul(qk_sb[:, p, :], f_[:], r[:])
            qk2 = spool.tile([128, M], f16, tag="qk2", name=f"qk2_{p}")
            nc.vector.tensor_scalar_mul(qk2[:], qk_sb[:, p, :],
                                        act_sb[:, p:p + 1])
            qk2_of[p] = qk2

        def emit_qkexp(p):
            """Q_p: broadcast qk2 over d via a 0-stride AP (ScalarE)."""
            qe = qpool.tile([128, F], f16, tag="qe", name=f"qe_{p}")
            qa = qk2_of[p][:]
            qk2b = bass.AP(tensor=qa.tensor, offset=qa.offset,
                           ap=[qa.ap[0], qa.ap[1], [0, D]])
            nc.scalar.copy(out=qe[:].rearrange("q (m d) -> q m d", d=D),
                           in_=qk2b)
            qe_of[p] = qe

        def emit_ncv(p):
            """N_p: qk-weighted accumulate (VectorE)."""
            votes, qe = votes_of[p], qe_of[p]
            if p == 0:
                nc.vector.tensor_mul(acc[:], votes[:], qe[:])
            else:
                prod2 = ppool.tile([128, F], f16, tag="prod",
                                   name=f"prod2_{p}")
                nc.vector.tensor_mul(prod2[:], votes[:], qe[:])
                nc.vector.tensor_add(acc[:], acc[:], prod2[:])

        # software pipeline over pairs, depth 2:
        #   DVE FIFO per cycle c: [L_c, S_{c-1}, N_{c-2}]
        #   ACT FIFO per cycle c: [E_{c+1}, Q_{c-1}, X_c]
        # so the 7us qk-broadcast Q_{c-1} overlaps L_c, and N_{c-2}'s input
        # was produced a full cycle earlier.
        emit_mms(0)
        emit_const_dmas()
        emit_mms(1)
        for c in range(NPAIR + 2):
            if 2 <= c + 1 <= NPAIR - 1:
                emit_mms(c + 1)
            if 0 <= c - 1 <= NPAIR - 1:
                emit_smalls(c - 1)
                emit_qkexp(c - 1)
            if c <= NPAIR - 1:
                emit_logits(c)
                emit_exp(c)
            if c >= 2:
                emit_ncv(c - 2)

        nc.sync.dma_start(out=qk_d[:], in_=qk_sb[:])
        nc.sync.dma_start(out=acc_d[:], in_=acc[:])

    nc.compile()
    return nc


def _in_maps(input, current_act, next_capsule_value, next_act, w):
    """Build the 8 per-core input maps (all fp16, pre-laid-out)."""
    f16 = np.float16
    input = np.asarray(input, np.float32)
    current_act = np.asarray(current_act, np.float32)
    ncv1 = np.asarray(next_capsule_value, np.float32).reshape(B, F).astype(f16)
    ncv = np.concatenate([ncv1, ncv1], axis=0)          # (128, F)
    na1 = np.asarray(next_act, np.float32).astype(f16)
    na = np.concatenate([na1, na1], axis=0)             # (128, M)
    w = np.asarray(w, np.float32)

    maps = []
    for c in range(NCORES):
        ns = slice(NLOC * c, NLOC * (c + 1))
        # w: (NLOC, A, M, D) -> (NGRP, 2, 128, F); rows 0:64 hold capsule
        # 4g+j's (a, m*d) slab, rows 64:128 hold capsule 4g+2+j's.
        wr = w[ns].reshape(NGRP, 4, A, F).astype(f16)
        w_dev = np.empty((NGRP, 2, 128, F), f16)
        w_dev[:, 0, :64] = wr[:, 0]
        w_dev[:, 0, 64:] = wr[:, 2]
        w_dev[:, 1, :64] = wr[:, 1]
        w_dev[:, 1, 64:] = wr[:, 3]
        # input: lhsT slabs (a, b) per capsule, same 4-capsule packing
        it = input[:, ns, :].transpose(2, 1, 0).reshape(A, NGRP, 4, B).astype(f16)
        in_dev = np.empty((128, NGRP, 2, B), f16)
        in_dev[:64, :, 0] = it[:, :, 0]
        in_dev[:64, :, 1] = it[:, :, 1]
        in_dev[64:, :, 0] = it[:, :, 2]
        in_dev[64:, :, 1] = it[:, :, 3]
        # current_act: per-partition gate per pair, partitions = (parity, b)
        ca = current_act[:, ns]                      # (B, NLOC)
        act_dev = np.empty((128, NPAIR), np.float32)
        act_dev[:64] = ca[:, 0::2]
        act_dev[64:] = ca[:, 1::2]
        maps.append({"w": w_dev, "inp": in_dev, "ncv": ncv, "na": na,
                     "act": act_dev})
    return maps


def _execute(maps, trace=False, **kw):
    from concourse.bass_utils import run_bass_kernel_spmd
    nc = _build()
    return run_bass_kernel_spmd(nc, maps, core_ids=list(range(NCORES)),
                                trace=trace, **kw)


def _gather(results):
    qk_full = np.empty((B, N, M), np.float32)
    ncv_out = np.zeros((B, M, D), np.float32)
    for c in range(NCORES):
        qk_c = np.asarray(results[c]["qk"], np.float32)     # (128, NPAIR, M)
        # partitions = (parity, b); local capsule index = 2*pair + parity
        qk_r = qk_c.reshape(2, 64, NPAIR, M).transpose(1, 2, 0, 3)
        qk_full[:, NLOC * c:NLOC * (c + 1), :] = qk_r.reshape(B, NLOC, M)
        a = np.asarray(results[c]["ncv_p"], np.float32).reshape(2, 64, M, D)
        ncv_out += a[0] + a[1]
    na_out = np.ones((B, M), np.float32)
    return ncv_out, na_out, qk_full


def kernel(input, current_act, next_capsule_value, next_act, w, num_iter=1):
    maps = _in_maps(input, current_act, next_capsule_value, next_act, w)
    res = _execute(maps)
    return _gather(res.results)


# revision 23
# speedup vs baseline: 1.0437x; 1.0064x over previous
"""CapsuleFC routing kernel for 8 Trainium2 NeuronCores.

Problem (B, N, A, M, D) = (64, 128, 64, 128, 64):
    votes  = einsum('bna,namd->bnmd', input, w)
    logits = einsum('bnmd,bmd->bnm', votes, ncv) * (1/sqrt(D))
    qk     = softmax(logits, axis=m) * next_act ; renormalized over m
    ncv'   = einsum('bnm,bnmd,bn->bmd', qk, votes, current_act)

Sharding: input capsules n are split 16-per-core (w is read exactly once
across the system).  Softmax over m is per-(b, n), so every stage is
core-local; the only cross-core reduction is the final sum over n of the
ncv' partials, done on host (8 tiny (64,128,64) adds).

Per-core layout: capsules are processed in 8 pairs.  SBUF/PSUM tensors use
partitions = (pair parity, batch) = 2*64 = 128, free = (m, d) = 8192.
votes come from TensorE matmuls (lhsT = input slice (a=64, b=64), moving =
w slice), packed 4 capsules at a time into the 128x128 PE array via
tile_position row/col groups.  ScalarE evicts PSUM->SBUF as fp16; VectorE
does the logits multiply + d-reduction tree, the (folded) softmax, and the
qk-weighted accumulation, all in fp16 2x mode.
"""

import functools
import sys

import numpy as np

if "/opt/trn_rl_repo" not in sys.path:
    sys.path.insert(0, "/opt/trn_rl_repo")

B, N, A, M, D = 64, 128, 64, 128, 64
NCORES = 8
NLOC = N // NCORES   # 16 capsules per core
NPAIR = NLOC // 2    # 8
NGRP = NLOC // 4     # 4 (matmul packing groups of 4 capsules)
F = M * D            # 8192
SCALE = 1.0 / np.sqrt(D)
EXP_BIAS = -2.0      # constant shift inside exp; cancels in renormalization


@functools.lru_cache(maxsize=1)
def _build():
    import concourse.tile as tile
    from concourse import bacc, mybir
    import concourse.bass as bass
    from contextlib import ExitStack

    f16 = mybir.dt.float16
    f32 = mybir.dt.float32
    mult = mybir.AluOpType.mult
    add = mybir.AluOpType.add

    nc = bacc.Bacc("TRN2", target_bir_lowering=False, debug=False,
                   num_devices=NCORES)

    w_d = nc.dram_tensor("w", [NGRP, 2, 128, F], f16, kind="ExternalInput")
    in_d = nc.dram_tensor("inp", [128, NGRP, 2, B], f16, kind="ExternalInput")
    ncv_d = nc.dram_tensor("ncv", [128, F], f16, kind="ExternalInput")
    na_d = nc.dram_tensor("na", [128, M], f16, kind="ExternalInput")
    act_d = nc.dram_tensor("act", [128, NPAIR], f32, kind="ExternalInput")
    qk_d = nc.dram_tensor("qk", [128, NPAIR, M], f32, kind="ExternalOutput")
    acc_d = nc.dram_tensor("ncv_p", [128, F], f16, kind="ExternalOutput")

    with tile.TileContext(nc) as tc, ExitStack() as ctx:
        consts = ctx.enter_context(tc.tile_pool(name="consts", bufs=1))
        wpool = ctx.enter_context(tc.tile_pool(name="wpool", bufs=8))
        vpool = ctx.enter_context(tc.tile_pool(name="votes", bufs=4))
        ppool = ctx.enter_context(tc.tile_pool(name="prod", bufs=1))
        qpool = ctx.enter_context(tc.tile_pool(name="qe", bufs=2))
        tpool = ctx.enter_context(tc.tile_pool(name="tree", bufs=1))
        spool = ctx.enter_context(tc.tile_pool(name="small", bufs=2))
        psum = ctx.enter_context(tc.tile_pool(name="psum", bufs=2, space="PSUM"))

        # --- constants / whole-kernel tensors ---
        in_sb = consts.tile([128, NGRP, 2, B], f16)
        nc.sync.dma_start(out=in_sb[:], in_=in_d[:])
        ncv_sb = consts.tile([128, F], f16)
        na_sb = consts.tile([128, M], f16)
        act_sb = consts.tile([128, NPAIR], f32)

        def emit_const_dmas():
            H = F // 2
            nc.sync.dma_start(out=ncv_sb[:, :H], in_=ncv_d[:, :H])
            nc.sync.dma_start(out=ncv_sb[:, H:], in_=ncv_d[:, H:])
            nc.sync.dma_start(out=na_sb[:], in_=na_d[:])
            nc.sync.dma_start(out=act_sb[:], in_=act_d[:])

        acc = consts.tile([128, F], f16)
        qk_sb = consts.tile([128, NPAIR, M], f32)
        exp_bias = consts.tile([128, 1], f32)
        nc.vector.memset(exp_bias[:], EXP_BIAS)

        w_tiles = {}
        CH = 2048           # psum tile free size (4 banks)
        NT = F // CH        # 4 psum tiles per pair

        def emit_w_dma(g):
            # one slab per (j, t): fine release granularity so the next
            # group's DMA streams in as this group's matmuls retire slabs
            for t in range(NT):
                for j in (0, 1):
                    s = wpool.tile([128, CH], f16, tag="w",
                                   name=f"w_{g}_{j}_{t}")
                    nc.sync.dma_start(
                        out=s[:], in_=w_d[g, j, :, t * CH:(t + 1) * CH])
                    w_tiles[(g, j, t)] = s

        votes_of = {}

        def emit_mms(p):
            """TensorE votes matmuls + ScalarE PSUM->SBUF eviction (E_p)."""
            g, s = p // 2, p % 2
            if s == 0 and (g, 0, 0) not in w_tiles:
                emit_w_dma(g)
            votes = vpool.tile([128, F], f16, tag="votes", name=f"votes_{p}")
            votes_of[p] = votes
            rows = slice(s * 64, s * 64 + 64)
            for t in range(NT):
                ps = psum.tile([128, CH], f32, tag="ps")
                w0, w1 = w_tiles[(g, 0, t)], w_tiles[(g, 1, t)]
                for cc in range(CH // 512):
                    col = cc * 512
                    nc.tensor.matmul(
                        ps[0:64, cc * 512:(cc + 1) * 512],
                        in_sb[rows, g, 0, :], w0[rows, col:col + 512],
                        start=True, stop=True, tile_position=(s * 64, 0))
                    nc.tensor.matmul(
                        ps[64:128, cc * 512:(cc + 1) * 512],
                        in_sb[rows, g, 1, :], w1[rows, col:col + 512],
                        start=True, stop=True, tile_position=(s * 64, 64))
                if p == 0 and t < 2:
                    nc.vector.tensor_copy(votes[:, t * CH:(t + 1) * CH],
                                          ps[:])
                else:
                    nc.scalar.copy(out=votes[:, t * CH:(t + 1) * CH],
                                   in_=ps[:])

        logit_of, e_of, sumexp_of, qk2_of, qe_of = {}, {}, {}, {}, {}

        def emit_logits(p):
            """L_p: votes*ncv multiply + d-reduction tree (VectorE)."""
            votes = votes_of[p]
            prod = ppool.tile([128, F], f16, tag="prod", name=f"prod_{p}")
            H = F // 2
            nc.vector.tensor_mul(prod[:, :H], votes[:, :H], ncv_sb[:, :H])
            nc.vector.tensor_mul(prod[:, H:], votes[:, H:], ncv_sb[:, H:])
            p3 = prod[:].rearrange("q (m d) -> q m d", d=D)
            t1 = tpool.tile([128, M, 32], f16, tag="t1")
            nc.vector.tensor_add(t1[:], p3[:, :, 0:32], p3[:, :, 32:64])
            t2 = tpool.tile([128, M, 16], f16, tag="t2")
            nc.vector.tensor_add(t2[:], t1[:, :, 0:16], t1[:, :, 16:32])
            t3 = tpool.tile([128, M, 8], f16, tag="t3")
            nc.vector.tensor_add(t3[:], t2[:, :, 0:8], t2[:, :, 8:16])
            t4 = tpool.tile([128, M, 4], f16, tag="t4")
            nc.vector.tensor_add(t4[:], t3[:, :, 0:4], t3[:, :, 4:8])
            t5 = tpool.tile([128, M, 2], f16, tag="t5")
            nc.vector.tensor_add(t5[:], t4[:, :, 0:2], t4[:, :, 2:4])
            logit = spool.tile([128, M], f32, tag="logit", name=f"logit_{p}")
            nc.vector.tensor_add(logit[:], t5[:, :, 0], t5[:, :, 1])
            logit_of[p] = logit

        def emit_exp(p):
            """X_p: exp (ScalarE)."""
            e = spool.tile([128, M], f16, tag="e", name=f"e_{p}")
            nc.scalar.activation(
                out=e[:], in_=logit_of[p][:],
                func=mybir.ActivationFunctionType.Exp,
                bias=exp_bias[:], scale=float(SCALE))
            e_of[p] = e

        def emit_smalls(p):
            """S_p: folded softmax renorm, qk output, gated qk (VectorE)."""
            e = e_of[p]
            f_ = spool.tile([128, M], f16, tag="f", name=f"f_{p}")
            s_ = spool.tile([128, 1], f32, tag="s", name=f"s_{p}")
            nc.vector.tensor_mul(f_[:], e[:], na_sb[:])
            nc.vector.tensor_reduce(out=s_[:], in_=f_[:],
                                    axis=mybir.AxisListType.X, op=add)
            r = spool.tile([128, 1], f32, tag="r", name=f"r_{p}")
            nc.vector.reciprocal(out=r[:], in_=s_[:])
            nc.vector.tensor_scalar_mul(qk_sb[:, p, :], f_[:], r[:])
            qk2 = spool.tile([128, M], f16, tag="qk2", name=f"qk2_{p}")
            nc.vector.tensor_scalar_mul(qk2[:], qk_sb[:, p, :],
                                        act_sb[:, p:p + 1])
            qk2_of[p] = qk2

        def emit_qkexp(p):
            """Q_p: broadcast qk2 over d via a 0-stride AP (ScalarE)."""
            qe = qpool.tile([128, F], f16, tag="qe", name=f"qe_{p}")
            qa = qk2_of[p][:]
            qk2b = bass.AP(tensor=qa.tensor, offset=qa.offset,
                           ap=[qa.ap[0], qa.ap[1], [0, D]])
            nc.scalar.copy(out=qe[:].rearrange("q (m d) -> q m d", d=D),
                           in_=qk2b)
            qe_of[p] = qe

        def emit_ncv(p):
            """N_p: qk-weighted accumulate (VectorE)."""
            votes = votes_of[p]
            if p == 0:
                nc.vector.tensor_mul(acc[:], votes[:], qe_of[p][:])
                return
            prod2 = ppool.tile([128, F], f16, tag="prod", name=f"prod2_{p}")
            if p == NPAIR - 1:
                qa = qk2_of[p][:]
                qk2b = bass.AP(tensor=qa.tensor, offset=qa.offset,
                               ap=[qa.ap[0], qa.ap[1], [0, D]])
                v3 = votes[:].rearrange("q (m d) -> q m d", d=D)
                nc.vector.tensor_mul(prod2[:].rearrange(
                    "q (m d) -> q m d", d=D), v3, qk2b)
            else:
                nc.vector.tensor_mul(prod2[:], votes[:], qe_of[p][:])
            nc.vector.tensor_add(acc[:], acc[:], prod2[:])

        # software pipeline over pairs, depth 2:
        #   DVE FIFO per cycle c: [L_c, S_{c-1}, N_{c-2}]
        #   ACT FIFO per cycle c: [E_{c+1}, Q_{c-1}, X_c]
        # so the 7us qk-broadcast Q_{c-1} overlaps L_c, and N_{c-2}'s input
        # was produced a full cycle earlier.
        # PE HAM warm-up: ~5us of junk matmuls while input DMAs stream in
        warm = consts.tile([128, 512], f16)
        nc.gpsimd.memset(warm[:], 0.5)
        warm_ps = psum.tile([128, 2048], f32, tag="ps")
        for _ in range(12):
            nc.tensor.matmul(warm_ps[:, 0:512], warm[:, 0:128], warm[:],
                             start=True, stop=True)

        emit_mms(0)
        emit_const_dmas()
        emit_mms(1)
        for c in range(NPAIR + 2):
            if 2 <= c + 1 <= NPAIR - 1:
                emit_mms(c + 1)
            if 0 <= c - 1 <= NPAIR - 1:
                emit_smalls(c - 1)
                if c - 1 < NPAIR - 1:
                    emit_qkexp(c - 1)
            if c <= NPAIR - 1:
                emit_logits(c)
                emit_exp(c)
            if c >= 2:
                emit_ncv(c - 2)

        nc.sync.dma_start(out=qk_d[:], in_=qk_sb[:])
        nc.sync.dma_start(out=acc_d[:], in_=acc[:])

    nc.compile()
    return nc


def _in_maps(input, current_act, next_capsule_value, next_act, w):
    """Build the 8 per-core input maps (all fp16, pre-laid-out)."""
    f16 = np.float16
    input = np.asarray(input, np.float32)
    current_act = np.asarray(current_act, np.float32)
    ncv1 = np.asarray(next_capsule_value, np.float32).reshape(B, F).astype(f16)
    ncv = np.concatenate([ncv1, ncv1], axis=0)          # (128, F)
    na1 = np.asarray(next_act, np.float32).astype(f16)
    na = np.concatenate([na1, na1], axis=0)             # (128, M)
    w = np.asarray(w, np.float32)

    maps = []
    for c in range(NCORES):
        ns = slice(NLOC * c, NLOC * (c + 1))
        # w: (NLOC, A, M, D) -> (NGRP, 2, 128, F); rows 0:64 hold capsule
        # 4g+j's (a, m*d) slab, rows 64:128 hold capsule 4g+2+j's.
        wr = w[ns].reshape(NGRP, 4, A, F).astype(f16)
        w_dev = np.empty((NGRP, 2, 128, F), f16)
        w_dev[:, 0, :64] = wr[:, 0]
        w_dev[:, 0, 64:] = wr[:, 2]
        w_dev[:, 1, :64] = wr[:, 1]
        w_dev[:, 1, 64:] = wr[:, 3]
        # input: lhsT slabs (a, b) per capsule, same 4-capsule packing
        it = input[:, ns, :].transpose(2, 1, 0).reshape(A, NGRP, 4, B).astype(f16)
        in_dev = np.empty((128, NGRP, 2, B), f16)
        in_dev[:64, :, 0] = it[:, :, 0]
        in_dev[:64, :, 1] = it[:, :, 1]
        in_dev[64:, :, 0] = it[:, :, 2]
        in_dev[64:, :, 1] = it[:, :, 3]
        # current_act: per-partition gate per pair, partitions = (parity, b)
        ca = current_act[:, ns]                      # (B, NLOC)
        act_dev = np.empty((128, NPAIR), np.float32)
        act_dev[:64] = ca[:, 0::2]
        act_dev[64:] = ca[:, 1::2]
        maps.append({"w": w_dev, "inp": in_dev, "ncv": ncv, "na": na,
                     "act": act_dev})
    return maps


def _execute(maps, trace=False, **kw):
    from concourse.bass_utils import run_bass_kernel_spmd
    nc = _build()
    return run_bass_kernel_spmd(nc, maps, core_ids=list(range(NCORES)),
                                trace=trace, **kw)


def _gather(results):
    qk_full = np.empty((B, N, M), np.float32)
    ncv_out = np.zeros((B, M, D), np.float32)
    for c in range(NCORES):
        qk_c = np.asarray(results[c]["qk"], np.float32)     # (128, NPAIR, M)
        # partitions = (parity, b); local capsule index = 2*pair + parity
        qk_r = qk_c.reshape(2, 64, NPAIR, M).transpose(1, 2, 0, 3)
        qk_full[:, NLOC * c:NLOC * (c + 1), :] = qk_r.reshape(B, NLOC, M)
        a = np.asarray(results[c]["ncv_p"], np.float32).reshape(2, 64, M, D)
        ncv_out += a[0] + a[1]
    na_out = np.ones((B, M), np.float32)
    return ncv_out, na_out, qk_full


def kernel(input, current_act, next_capsule_value, next_act, w, num_iter=1):
    maps = _in_maps(input, current_act, next_capsule_value, next_act, w)
    res = _execute(maps)
    return _gather(res.results)
